# revision 2
# baseline (speedup 1.0000x reference)
"""Trainium2 Bass kernel for nn_Decoder_32074815767263 (dense_mlp).

Math (per reference):
    enc_proj = enc_state @ W1[:512]          (B,T,H)
    dec_proj = dec_state @ W1[512:]          (B,U,H)
    hidden   = tanh(enc_proj[:,:,None,:] + dec_proj[:,None,:,:] + b1)
    logits   = hidden @ W2 + b2              (B,T,U,V)

Sharding: 8 cores = (B=4) x (U halves of 30). Each core computes its
(300, 30, 1000) output slab independently; no collectives.

Per-core pipeline (SPMD-identical program, data differs per core):
  - input DMA split across both HWDGE rings (SP + ACT) so weights land
    in ~half the serialized time; PE warmup matmuls on a zero tile keep
    the PE HAM at 2.4 GHz through the DMA-bound startup.
  - enc_projT[h,t] / dec_projT[h,u] via bf16 matmuls into fp32 PSUM.
  - hiddenT materialized transposed [H-part, row], row = u*300+t, into a
    4608-column circular SBUF buffer (bf16):
      adds: hid[:,h,span] = encp (bf16) + dec_projT[h,u]+b1 scalar,
            h 0-3 on DVE, h 4-7 on GpSimd
      tanh: ACT, groups of 300/600/768 cols (small first groups unblock
            the first matmul blocks early)
  - PE: per 128-row block, 16 bf16 matmuls (8 H-chunks x 2 vocab halves)
    accumulating into a 2-bank PSUM tile.
  - DVE: single-op drain psum + b2 -> fp32 out tile; stores alternate
    between the SP and ACT HWDGE rings (2:1).
"""

import sys

for _p in ("/opt/trn_rl_repo", "/root/.axon_site/_ro/trn_rl_repo"):
    if _p not in sys.path:
        sys.path.append(_p)

import ml_dtypes
import numpy as np

_B, _T, _U = 4, 300, 60
_D, _H, _V = 512, 1024, 1000
_UC = 30                       # U cols per core
_ROWS = _T * _UC               # 9000 hidden rows per core
_CB = 4608                     # circular hid buffer columns (mult of 128 & 768)
_NBLK = (_ROWS + 127) // 128   # 71 matmul row-blocks
_WARM1 = 26                    # PE warmup matmuls before projections
_WARM2 = 6                     # PE warmup matmuls between proj and block 0

_PROGRAM = None


def _tanh_boundaries():
    bs = [300, 900]
    while bs[-1] < _ROWS:
        bs.append(min(bs[-1] + 768, _ROWS))
    return bs


def _build_program():
    from contextlib import ExitStack

    import concourse.bass as bass
    import concourse.tile as tile
    from concourse import bacc, mybir

    f32 = mybir.dt.float32
    bf16 = mybir.dt.bfloat16
    Tanh = mybir.ActivationFunctionType.Tanh

    nc = bacc.Bacc("TRN2", target_bir_lowering=False, debug=False)

    encT = nc.dram_tensor("encT", [_D, _T], bf16, kind="ExternalInput")
    decT = nc.dram_tensor("decT", [_D, _UC], bf16, kind="ExternalInput")
    W1 = nc.dram_tensor("W1", [2 * _D, _H], bf16, kind="ExternalInput")
    b1T = nc.dram_tensor("b1T", [128, 8], f32, kind="ExternalInput")
    W2 = nc.dram_tensor("W2", [_H, _V], bf16, kind="ExternalInput")
    b2b = nc.dram_tensor("b2b", [128, _V], f32, kind="ExternalInput")
    out = nc.dram_tensor("out", [_T, _UC, _V], f32, kind="ExternalOutput")

    with ExitStack() as ctx:
        tc = ctx.enter_context(tile.TileContext(nc))
        consts = ctx.enter_context(tc.tile_pool(name="consts", bufs=1))
        outp = ctx.enter_context(tc.tile_pool(name="outp", bufs=6))
        psmain = ctx.enter_context(tc.tile_pool(name="psmain", bufs=2, space="PSUM"))
        psdum = ctx.enter_context(tc.tile_pool(name="psdum", bufs=1, space="PSUM"))
        psproj = ctx.enter_context(tc.tile_pool(name="psproj", bufs=3, space="PSUM"))

        w1t = consts.tile([128, 8, _H], bf16, tag="w1t")
        w2t = consts.tile([128, 8, _V], bf16, tag="w2t")
        encTs = consts.tile([128, 4, _T], bf16, tag="encTs")
        decTs = consts.tile([128, 4, _UC], bf16, tag="decTs")
        b1s = consts.tile([128, 8], f32, tag="b1s")
        b2s = consts.tile([128, _V], f32, tag="b2s")
        encp = consts.tile([128, 8, _T], bf16, tag="encp")
        dpb = consts.tile([128, 8, _UC], f32, tag="dpb")
        hid = consts.tile([128, 8, _CB], bf16, tag="hid")
        warm = consts.tile([128, 512], bf16, tag="warm")

        nc.vector.memset(warm[:], 0.0)

        # ---- input DMA: split across the two HWDGE rings, ordered by first use
        W1r = W1[:].rearrange("(c p) h -> p c h", p=128)
        W2r = W2[:].rearrange("(c p) v -> p c v", p=128)
        # SP ring: dec-proj deps
        nc.sync.dma_start(out=decTs[:], in_=decT[:].rearrange("(c p) t -> p c t", p=128))
        nc.sync.dma_start(out=b1s[:], in_=b1T[:])
        for d in range(4, 8):
            nc.sync.dma_start(out=w1t[:, d, :], in_=W1r[:, d, :])
        # ACT ring: enc-proj deps, then W2 (needed from block 0 on), then b2
        nc.scalar.dma_start(out=encTs[:], in_=encT[:].rearrange("(c p) t -> p c t", p=128))
        for d in range(4):
            nc.scalar.dma_start(out=w1t[:, d, :], in_=W1r[:, d, :])
        for c in range(8):
            nc.scalar.dma_start(out=w2t[:, c, :], in_=W2r[:, c, :])
        nc.scalar.dma_start(out=b2s[:], in_=b2b[:])

        # ---- PE warmup: HAM needs ~3.4us of activity to clock up; the
        # startup is DMA-bound, so burn idle PE time on dummy matmuls.
        def emit_warm(n, psd_tile):
            for _ in range(n):
                nc.tensor.matmul(
                    psd_tile[:, 0:500], warm[:, 0:128], warm[:, 0:500],
                    start=True, stop=True,
                )

        dum = psdum.tile([128, 512], f32, tag="dum")
        emit_warm(_WARM1, dum)

        # ---- projections (dec first: dpb feeds every add) ----
        psd = psproj.tile([128, 8, 32], f32, tag="pp")
        for h in range(8):
            for d in range(4):
                nc.tensor.matmul(
                    psd[:, h, 0:_UC],
                    w1t[:, 4 + d, 128 * h : 128 * (h + 1)],
                    decTs[:, d, :],
                    start=(d == 0),
                    stop=(d == 3),
                )
        for h in range(8):
            nc.vector.tensor_scalar_add(
                out=dpb[:, h, :], in0=psd[:, h, 0:_UC], scalar1=b1s[:, h : h + 1]
            )
        for h in range(8):
            pse = psproj.tile([128, 304], f32, tag="pp")
            for d in range(4):
                nc.tensor.matmul(
                    pse[:, 0:_T],
                    w1t[:, d, 128 * h : 128 * (h + 1)],
                    encTs[:, d, :],
                    start=(d == 0),
                    stop=(d == 3),
                )
            nc.vector.tensor_copy(out=encp[:, h, :], in_=pse[:, 0:_T])

        emit_warm(_WARM2, dum)

        # ---- pre-activation adds: h 0-3 on DVE, h 4-7 on GpSimd ----
        def emit_add(u):
            off = (_T * u) % _CB
            L = min(_T, _CB - off)
            for h in range(8):
                eng = nc.vector if h < 4 else nc.gpsimd
                eng.tensor_scalar_add(
                    out=hid[:, h, off : off + L],
                    in0=encp[:, h, 0:L],
                    scalar1=dpb[:, h, u : u + 1],
                )
                if L < _T:
                    eng.tensor_scalar_add(
                        out=hid[:, h, 0 : _T - L],
                        in0=encp[:, h, L:_T],
                        scalar1=dpb[:, h, u : u + 1],
                    )

        # ---- per-block matmul + drain + store ----
        def emit_block(k):
            r0 = 128 * k
            M = min(128, _ROWS - r0)
            c0 = r0 % _CB
            ps = psmain.tile([128, 2, 512], f32, tag="ps")
            for v in range(2):
                for h in range(8):
                    nc.tensor.matmul(
                        ps[:M, v, 0:500],
                        hid[:, h, c0 : c0 + M],
                        w2t[:, h, 500 * v : 500 * (v + 1)],
                        start=(h == 0),
                        stop=(h == 7),
                    )
            ot = outp.tile([128, _V], f32, tag="ot")
            nc.vector.tensor_add(
                out=ot[:M].rearrange("p (v n) -> p v n", v=2),
                in0=ps[:M, :, 0:500],
                in1=b2s[:M].rearrange("p (v n) -> p v n", v=2),
            )
            ring = nc.sync if k % 3 < 2 else nc.scalar
            u0, t0 = divmod(r0, _T)
            if t0 + M <= _T:
                ring.dma_start(out=out[t0 : t0 + M, u0, :], in_=ot[:M, :])
            else:
                L = _T - t0
                ring.dma_start(out=out[t0:_T, u0, :], in_=ot[0:L, :])
                ring.dma_start(out=out[0 : M - L, u0 + 1, :], in_=ot[L:M, :])

        # ---- main loop: produce u-tiles, tanh groups, consume blocks ----
        bounds = _tanh_boundaries()
        next_g = 0
        next_blk = 0
        tanh_prev = 0

        def emit_tanh(a, b):
            base = a % _CB
            L = b - a
            for h in range(8):
                if base + L <= _CB:
                    nc.scalar.activation(
                        out=hid[:, h, base : base + L],
                        in_=hid[:, h, base : base + L],
                        func=Tanh,
                    )
                else:
                    L1 = _CB - base
                    nc.scalar.activation(
                        out=hid[:, h, base:_CB], in_=hid[:, h, base:_CB], func=Tanh
                    )
                    nc.scalar.activation(
                        out=hid[:, h, 0 : L - L1], in_=hid[:, h, 0 : L - L1], func=Tanh
                    )

        for u in range(_UC):
            emit_add(u)
            done = _T * (u + 1)
            while next_g < len(bounds) and bounds[next_g] <= done:
                emit_tanh(tanh_prev, bounds[next_g])
                tanh_prev = bounds[next_g]
                next_g += 1
                while next_blk < _NBLK and min(128 * (next_blk + 1), _ROWS) <= tanh_prev:
                    emit_block(next_blk)
                    next_blk += 1
        assert next_g == len(bounds) and next_blk == _NBLK, (next_g, next_blk)

    nc.finalize()
    return nc


def _get_program():
    global _PROGRAM
    if _PROGRAM is None:
        _PROGRAM = _build_program()
    return _PROGRAM


def _make_in_maps(enc, dec, W1, b1, W2, b2):
    bf = ml_dtypes.bfloat16
    b1T = np.ascontiguousarray(b1.reshape(8, 128).T)
    b2b = np.ascontiguousarray(np.broadcast_to(b2, (128, _V)))
    W1b = W1.astype(bf)
    W2b = W2.astype(bf)
    in_maps = []
    for c in range(8):
        b, half = divmod(c, 2)
        in_maps.append(
            {
                "encT": np.ascontiguousarray(enc[b].T.astype(bf)),
                "decT": np.ascontiguousarray(
                    dec[b, half * _UC : (half + 1) * _UC, :].T.astype(bf)
                ),
                "W1": W1b,
                "b1T": b1T,
                "W2": W2b,
                "b2b": b2b,
            }
        )
    return in_maps


def kernel(enc_state, dec_state, W1, b1, W2, b2):
    from concourse.bass_utils import run_bass_kernel_spmd

    enc = np.ascontiguousarray(np.asarray(enc_state, dtype=np.float32))
    dec = np.ascontiguousarray(np.asarray(dec_state, dtype=np.float32))
    W1 = np.ascontiguousarray(np.asarray(W1, dtype=np.float32))
    b1 = np.ascontiguousarray(np.asarray(b1, dtype=np.float32))
    W2 = np.ascontiguousarray(np.asarray(W2, dtype=np.float32))
    b2 = np.ascontiguousarray(np.asarray(b2, dtype=np.float32))

    nc = _get_program()
    in_maps = _make_in_maps(enc, dec, W1, b1, W2, b2)
    res = run_bass_kernel_spmd(nc, in_maps, list(range(8)))

    full = np.empty((_B, _T, _U, _V), np.float32)
    for c in range(8):
        b, half = divmod(c, 2)
        full[b, :, half * _UC : (half + 1) * _UC, :] = res.results[c]["out"]
    return full


# revision 3
# speedup vs baseline: 2.8680x; 2.8680x over previous
"""Trainium2 Bass kernel for nn_Decoder_32074815767263 (dense_mlp).

Math (per reference):
    enc_proj = enc_state @ W1[:512]          (B,T,H)
    dec_proj = dec_state @ W1[512:]          (B,U,H)
    hidden   = tanh(enc_proj[:,:,None,:] + dec_proj[:,None,:,:] + b1)
    logits   = hidden @ W2 + b2              (B,T,U,V)

Sharding: 8 cores = (B=4) x (U halves of 30). Each core computes its
(300, 30, 1000) output slab independently; no collectives.

Per-core pipeline (SPMD-identical program, data differs per core):
  - input DMA split across both HWDGE rings (SP + ACT) so weights land
    in ~half the serialized time; PE warmup matmuls on a zero tile keep
    the PE HAM at 2.4 GHz through the DMA-bound startup.
  - enc_projT[h,t] / dec_projT[h,u] via bf16 matmuls into fp32 PSUM.
  - hiddenT materialized transposed [H-part, row], row = u*300+t, into a
    4608-column circular SBUF buffer (bf16):
      adds: hid[:,h,span] = encp (bf16) + dec_projT[h,u]+b1 scalar,
            h 0-3 on DVE, h 4-7 on GpSimd
      tanh: ACT, groups of 300/600/768 cols (small first groups unblock
            the first matmul blocks early)
  - PE: per 128-row block, 16 bf16 matmuls (8 H-chunks x 2 vocab halves)
    accumulating into a 2-bank PSUM tile.
  - DVE: single-op drain psum + b2 -> fp32 out tile; stores alternate
    between the SP and ACT HWDGE rings (2:1).
"""

import sys

for _p in ("/opt/trn_rl_repo", "/root/.axon_site/_ro/trn_rl_repo"):
    if _p not in sys.path:
        sys.path.append(_p)

import ml_dtypes
import numpy as np

_B, _T, _U = 4, 300, 60
_D, _H, _V = 512, 1024, 1000
_UC = 30                       # U cols per core
_ROWS = _T * _UC               # 9000 hidden rows per core
_CB = 4608                     # circular hid buffer columns (mult of 128 & 768)
_NBLK = (_ROWS + 127) // 128   # 71 matmul row-blocks
_WARM1 = 26                    # PE warmup matmuls before projections
_WARM2 = 6                     # PE warmup matmuls between proj and block 0

_PROGRAM = None


def _tanh_boundaries():
    bs = [300, 900]
    while bs[-1] < _ROWS:
        bs.append(min(bs[-1] + 768, _ROWS))
    return bs


def _build_program():
    from contextlib import ExitStack

    import concourse.bass as bass
    import concourse.tile as tile
    from concourse import bacc, mybir

    f32 = mybir.dt.float32
    bf16 = mybir.dt.bfloat16
    Tanh = mybir.ActivationFunctionType.Tanh

    nc = bacc.Bacc("TRN2", target_bir_lowering=False, debug=False)

    encT = nc.dram_tensor("encT", [_D, _T], bf16, kind="ExternalInput")
    decT = nc.dram_tensor("decT", [_D, _UC], bf16, kind="ExternalInput")
    W1 = nc.dram_tensor("W1", [2 * _D, _H], bf16, kind="ExternalInput")
    b1T = nc.dram_tensor("b1T", [128, 8], f32, kind="ExternalInput")
    W2 = nc.dram_tensor("W2", [_H, _V], bf16, kind="ExternalInput")
    b2b = nc.dram_tensor("b2b", [128, _V], f32, kind="ExternalInput")
    out = nc.dram_tensor("out", [_T, _UC, _V], f32, kind="ExternalOutput")

    with ExitStack() as ctx:
        tc = ctx.enter_context(tile.TileContext(nc))
        consts = ctx.enter_context(tc.tile_pool(name="consts", bufs=1))
        outp = ctx.enter_context(tc.tile_pool(name="outp", bufs=6))
        psmain = ctx.enter_context(tc.tile_pool(name="psmain", bufs=2, space="PSUM"))
        psdum = ctx.enter_context(tc.tile_pool(name="psdum", bufs=1, space="PSUM"))
        psproj = ctx.enter_context(tc.tile_pool(name="psproj", bufs=3, space="PSUM"))

        w1t = consts.tile([128, 8, _H], bf16, tag="w1t")
        w2t = consts.tile([128, 8, _V], bf16, tag="w2t")
        encTs = consts.tile([128, 4, _T], bf16, tag="encTs")
        decTs = consts.tile([128, 4, _UC], bf16, tag="decTs")
        b1s = consts.tile([128, 8], f32, tag="b1s")
        b2s = consts.tile([128, _V], f32, tag="b2s")
        encp = consts.tile([128, 8, _T], bf16, tag="encp")
        dpb = consts.tile([128, 8, _UC], f32, tag="dpb")
        hid = consts.tile([128, 8, _CB], bf16, tag="hid")
        warm = consts.tile([128, 512], bf16, tag="warm")

        nc.vector.memset(warm[:], 0.0)

        # ---- input DMA: split across the two HWDGE rings, ordered by first use
        W1r = W1[:].rearrange("(c p) h -> p c h", p=128)
        W2r = W2[:].rearrange("(c p) v -> p c v", p=128)
        # SP ring: dec-proj deps
        nc.sync.dma_start(out=decTs[:], in_=decT[:].rearrange("(c p) t -> p c t", p=128))
        nc.sync.dma_start(out=b1s[:], in_=b1T[:])
        for d in range(4, 8):
            nc.sync.dma_start(out=w1t[:, d, :], in_=W1r[:, d, :])
        # ACT ring: enc-proj deps, then W2 (needed from block 0 on), then b2
        nc.scalar.dma_start(out=encTs[:], in_=encT[:].rearrange("(c p) t -> p c t", p=128))
        for d in range(4):
            nc.scalar.dma_start(out=w1t[:, d, :], in_=W1r[:, d, :])
        for c in range(8):
            nc.scalar.dma_start(out=w2t[:, c, :], in_=W2r[:, c, :])
        nc.scalar.dma_start(out=b2s[:], in_=b2b[:])

        # ---- PE warmup: HAM needs ~3.4us of activity to clock up; the
        # startup is DMA-bound, so burn idle PE time on dummy matmuls.
        def emit_warm(n, psd_tile):
            for _ in range(n):
                nc.tensor.matmul(
                    psd_tile[:, 0:500], warm[:, 0:128], warm[:, 0:500],
                    start=True, stop=True,
                )

        dum = psdum.tile([128, 512], f32, tag="dum")
        emit_warm(_WARM1, dum)

        # ---- projections (dec first: dpb feeds every add) ----
        psd = psproj.tile([128, 8, 32], f32, tag="pp")
        for h in range(8):
            for d in range(4):
                nc.tensor.matmul(
                    psd[:, h, 0:_UC],
                    w1t[:, 4 + d, 128 * h : 128 * (h + 1)],
                    decTs[:, d, :],
                    start=(d == 0),
                    stop=(d == 3),
                )
        for h in range(8):
            nc.vector.tensor_scalar_add(
                out=dpb[:, h, :], in0=psd[:, h, 0:_UC], scalar1=b1s[:, h : h + 1]
            )
        for h in range(8):
            pse = psproj.tile([128, 304], f32, tag="pp")
            for d in range(4):
                nc.tensor.matmul(
                    pse[:, 0:_T],
                    w1t[:, d, 128 * h : 128 * (h + 1)],
                    encTs[:, d, :],
                    start=(d == 0),
                    stop=(d == 3),
                )
            nc.vector.tensor_copy(out=encp[:, h, :], in_=pse[:, 0:_T])

        emit_warm(_WARM2, dum)

        # ---- pre-activation adds (DVE only: GpSimd shares the DVE SBUF
        # port and starves it; DVE hits the 4x perf mode at ~108ns/op) ----
        def emit_add(u):
            off = (_T * u) % _CB
            L = min(_T, _CB - off)
            for h in range(8):
                eng = nc.vector
                eng.tensor_scalar_add(
                    out=hid[:, h, off : off + L],
                    in0=encp[:, h, 0:L],
                    scalar1=dpb[:, h, u : u + 1],
                )
                if L < _T:
                    eng.tensor_scalar_add(
                        out=hid[:, h, 0 : _T - L],
                        in0=encp[:, h, L:_T],
                        scalar1=dpb[:, h, u : u + 1],
                    )

        # ---- per-block matmul + drain + store ----
        def emit_block(k):
            r0 = 128 * k
            M = min(128, _ROWS - r0)
            c0 = r0 % _CB
            ps = psmain.tile([128, 2, 512], f32, tag="ps")
            for v in range(2):
                for h in range(8):
                    nc.tensor.matmul(
                        ps[:M, v, 0:500],
                        hid[:, h, c0 : c0 + M],
                        w2t[:, h, 500 * v : 500 * (v + 1)],
                        start=(h == 0),
                        stop=(h == 7),
                    )
            ot = outp.tile([128, _V], f32, tag="ot")
            nc.vector.tensor_add(
                out=ot[:M].rearrange("p (v n) -> p v n", v=2),
                in0=ps[:M, :, 0:500],
                in1=b2s[:M].rearrange("p (v n) -> p v n", v=2),
            )
            ring = nc.sync if k % 3 < 2 else nc.scalar
            u0, t0 = divmod(r0, _T)
            if t0 + M <= _T:
                ring.dma_start(out=out[t0 : t0 + M, u0, :], in_=ot[:M, :])
            else:
                L = _T - t0
                ring.dma_start(out=out[t0:_T, u0, :], in_=ot[0:L, :])
                ring.dma_start(out=out[0 : M - L, u0 + 1, :], in_=ot[L:M, :])

        # ---- main loop: produce u-tiles, tanh groups, consume blocks ----
        bounds = _tanh_boundaries()
        next_g = 0
        next_blk = 0
        tanh_prev = 0

        def emit_tanh(a, b):
            base = a % _CB
            L = b - a
            for h in range(8):
                if base + L <= _CB:
                    nc.scalar.activation(
                        out=hid[:, h, base : base + L],
                        in_=hid[:, h, base : base + L],
                        func=Tanh,
                    )
                else:
                    L1 = _CB - base
                    nc.scalar.activation(
                        out=hid[:, h, base:_CB], in_=hid[:, h, base:_CB], func=Tanh
                    )
                    nc.scalar.activation(
                        out=hid[:, h, 0 : L - L1], in_=hid[:, h, 0 : L - L1], func=Tanh
                    )

        for u in range(_UC):
            emit_add(u)
            done = _T * (u + 1)
            while next_g < len(bounds) and bounds[next_g] <= done:
                emit_tanh(tanh_prev, bounds[next_g])
                tanh_prev = bounds[next_g]
                next_g += 1
                while next_blk < _NBLK and min(128 * (next_blk + 1), _ROWS) <= tanh_prev:
                    emit_block(next_blk)
                    next_blk += 1
        assert next_g == len(bounds) and next_blk == _NBLK, (next_g, next_blk)

    nc.finalize()
    return nc


def _get_program():
    global _PROGRAM
    if _PROGRAM is None:
        _PROGRAM = _build_program()
    return _PROGRAM


def _make_in_maps(enc, dec, W1, b1, W2, b2):
    bf = ml_dtypes.bfloat16
    b1T = np.ascontiguousarray(b1.reshape(8, 128).T)
    b2b = np.ascontiguousarray(np.broadcast_to(b2, (128, _V)))
    W1b = W1.astype(bf)
    W2b = W2.astype(bf)
    in_maps = []
    for c in range(8):
        b, half = divmod(c, 2)
        in_maps.append(
            {
                "encT": np.ascontiguousarray(enc[b].T.astype(bf)),
                "decT": np.ascontiguousarray(
                    dec[b, half * _UC : (half + 1) * _UC, :].T.astype(bf)
                ),
                "W1": W1b,
                "b1T": b1T,
                "W2": W2b,
                "b2b": b2b,
            }
        )
    return in_maps


def kernel(enc_state, dec_state, W1, b1, W2, b2):
    from concourse.bass_utils import run_bass_kernel_spmd

    enc = np.ascontiguousarray(np.asarray(enc_state, dtype=np.float32))
    dec = np.ascontiguousarray(np.asarray(dec_state, dtype=np.float32))
    W1 = np.ascontiguousarray(np.asarray(W1, dtype=np.float32))
    b1 = np.ascontiguousarray(np.asarray(b1, dtype=np.float32))
    W2 = np.ascontiguousarray(np.asarray(W2, dtype=np.float32))
    b2 = np.ascontiguousarray(np.asarray(b2, dtype=np.float32))

    nc = _get_program()
    in_maps = _make_in_maps(enc, dec, W1, b1, W2, b2)
    res = run_bass_kernel_spmd(nc, in_maps, list(range(8)))

    full = np.empty((_B, _T, _U, _V), np.float32)
    for c in range(8):
        b, half = divmod(c, 2)
        full[b, :, half * _UC : (half + 1) * _UC, :] = res.results[c]["out"]
    return full
